# revision 1
# baseline (speedup 1.0000x reference)
"""Block-Hadamard transform kernel for Trainium2 (8 NeuronCores).

y[b, s, g*128:(g+1)*128] = x[b, s, g*128:(g+1)*128] @ H   for each 128-block g,
with H a 128x128 (symmetric, orthogonal) Hadamard matrix.

Strategy (data parallel over rows = batch*seq, no communication):
  - Each core gets ROWS/8 = 2048 rows of [4096] f32.
  - Per [128, 4096] SBUF tile (rows on partitions, natural DMA layout):
      for each 128-wide block g:
        1. PE transpose:   xT_g = x_g.T           (via identity matmul -> PSUM)
        2. DVE copy:       xT_g PSUM -> SBUF
        3. PE matmul:      y_g = matmul(lhsT=xT_g, rhs=H) = x_g @ H -> PSUM
           (output lands in NATURAL [row, k] layout -> no transpose-back)
        4. ACT copy:       y_g PSUM -> SBUF output tile
  - One 2 MiB in-DMA (SP HWDGE ring) and two 1 MiB out-DMAs (ACT HWDGE
    ring) per [128, 4096] tile; input loads software-pipelined one tile
    ahead; ~4us PE warm-up loop keeps the HAM clock gate at 8/8.
  Measured: ~200us HW exec/core (DMA roofline ~188us), rel err ~2e-7.
"""

import sys

for _p in ("/opt/trn_rl_repo", "/opt/pypackages"):
    if _p not in sys.path:
        sys.path.insert(0, _p)

import numpy as np

import concourse.bass as bass
import concourse.mybir as mybir
import concourse.tile as tile
from concourse import bacc
from concourse.bass_utils import run_bass_kernel_spmd

N_CORES = 8
BSZ, SEQ, EMB = 4, 4096, 4096
HS = 128
P = 128
ROWS = BSZ * SEQ                 # 16384
ROWS_PER_CORE = ROWS // N_CORES  # 2048
N_TILES = ROWS_PER_CORE // P     # 16
GRP = 512                        # columns per PSUM group (4 blocks, 1 bank)
N_GRPS = EMB // GRP              # 8
BLK_PER_GRP = GRP // 128         # 4

_cached_nc = None
_IDENT = np.eye(128, dtype=np.float32)

# Set by test.py for profiling; harness path leaves these alone.
TRACE = False
LAST_RESULT = None


def _build():
    nc = bacc.Bacc("TRN2", target_bir_lowering=False, debug=False)
    x = nc.dram_tensor(
        "x", [ROWS_PER_CORE, EMB], mybir.dt.float32, kind="ExternalInput"
    ).ap()
    h = nc.dram_tensor("h", [HS, HS], mybir.dt.float32, kind="ExternalInput").ap()
    idm = nc.dram_tensor(
        "idm", [P, P], mybir.dt.float32, kind="ExternalInput"
    ).ap()
    y = nc.dram_tensor(
        "y", [ROWS_PER_CORE, EMB], mybir.dt.float32, kind="ExternalOutput"
    ).ap()

    with tile.TileContext(nc) as tc:
        with (
            tc.tile_pool(name="const", bufs=1) as const_pool,
            tc.tile_pool(name="xin", bufs=4) as xin_pool,
            tc.tile_pool(name="yout", bufs=3) as yout_pool,
            tc.tile_pool(name="xT", bufs=6) as xT_pool,
            tc.tile_pool(name="psA", bufs=4, space="PSUM") as psA_pool,
            tc.tile_pool(name="psB", bufs=4, space="PSUM") as psB_pool,
        ):
            h_sb = const_pool.tile([HS, HS], mybir.dt.float32)
            nc.sync.dma_start(h_sb[:], h)
            # Identity comes in via DMA: building it with gpsimd
            # (memset+affine_select) forces ~17us of GPSIMD library
            # TENSOR_LOADs into the kernel preamble.
            ident = const_pool.tile([P, P], mybir.dt.float32)
            nc.sync.dma_start(ident[:], idm)

            # PE warmups: make PE observe the producers of ident (gpsimd) and
            # h_sb (DMA) before the main loop; reduces steady-state waits and
            # pre-warms HAM slightly. Tags shared with the loop tiles so the
            # PSUM pools don't allocate extra slots.
            w1 = psA_pool.tile([P, GRP], mybir.dt.float32, tag="ps_xT")
            nc.tensor.transpose(w1[:, 0:128], ident[:], ident[:])
            w2 = psB_pool.tile([P, GRP], mybir.dt.float32, tag="ps_y")
            nc.tensor.matmul(w2[:, 0:128], h_sb[:], h_sb[:], start=True, stop=True)
            # HAM warm-up: ~4us of dummy PE activity while the first input
            # tile is still streaming in, so the clock gate is already at
            # 8/8 when real work starts (it needs ~3.4us of sustained PE
            # busy to unthrottle from the cold 4/8 state).
            for _ in range(24):
                nc.tensor.transpose(w1[:, 0:128], ident[:], ident[:])

            # Software-pipelined input prefetch: the in-DMA for tile t+1 is
            # emitted BEFORE tile t's compute so the scheduler prioritizes
            # keeping the PE fed (PE stalls re-throttle the HAM clock gate).
            # First tile's load is split in quarters so the PE can start on
            # the first 1024 columns ~4x sooner (startup bubble).
            xt_next = xin_pool.tile([P, EMB], mybir.dt.float32, tag="xt")
            for q in range(4):
                nc.sync.dma_start(
                    xt_next[:, q * 1024 : (q + 1) * 1024],
                    x[0:P, q * 1024 : (q + 1) * 1024],
                )
            for t in range(N_TILES):
                xt = xt_next
                if t + 1 < N_TILES:
                    xt_next = xin_pool.tile([P, EMB], mybir.dt.float32, tag="xt")
                    nc.sync.dma_start(
                        xt_next[:], x[(t + 1) * P : (t + 2) * P, :]
                    )
                yt = yout_pool.tile([P, EMB], mybir.dt.float32)
                for g in range(N_GRPS):
                    ps_xT = psA_pool.tile([P, GRP], mybir.dt.float32)
                    for b in range(BLK_PER_GRP):
                        c0 = g * GRP + b * 128
                        nc.tensor.transpose(
                            ps_xT[:, b * 128 : (b + 1) * 128],
                            xt[:, c0 : c0 + 128],
                            ident[:],
                        )
                    xT_sb = xT_pool.tile([P, GRP], mybir.dt.float32)
                    nc.vector.tensor_copy(xT_sb[:], ps_xT[:])
                    ps_y = psB_pool.tile([P, GRP], mybir.dt.float32)
                    for b in range(BLK_PER_GRP):
                        nc.tensor.matmul(
                            ps_y[:, b * 128 : (b + 1) * 128],
                            xT_sb[:, b * 128 : (b + 1) * 128],
                            h_sb[:],
                            start=True,
                            stop=True,
                        )
                    nc.scalar.copy(yt[:, g * GRP : (g + 1) * GRP], ps_y[:])
                # Out-DMAs go through the second HWDGE ring (ACT engine) so
                # input loads on the SP ring never queue behind them; the
                # SDMA engines round-robin between the two queues at packet
                # granularity. Split in halves for finer interleave.
                nc.scalar.dma_start(
                    y[t * P : (t + 1) * P, 0 : EMB // 2], yt[:, 0 : EMB // 2]
                )
                nc.scalar.dma_start(
                    y[t * P : (t + 1) * P, EMB // 2 : EMB], yt[:, EMB // 2 : EMB]
                )
    nc.compile()
    return nc


def kernel(hidden_states, H):
    global _cached_nc, LAST_RESULT
    hs = np.ascontiguousarray(np.asarray(hidden_states, dtype=np.float32)).reshape(
        ROWS, EMB
    )
    Hm = np.ascontiguousarray(np.asarray(H, dtype=np.float32))
    if _cached_nc is None:
        _cached_nc = _build()
    nc = _cached_nc
    in_maps = [
        {
            "x": hs[i * ROWS_PER_CORE : (i + 1) * ROWS_PER_CORE],
            "h": Hm,
            "idm": _IDENT,
        }
        for i in range(N_CORES)
    ]
    res = run_bass_kernel_spmd(
        nc, in_maps, core_ids=list(range(N_CORES)), trace=TRACE
    )
    LAST_RESULT = res
    out = np.concatenate([r["y"] for r in res.results], axis=0)
    return out.reshape(BSZ, SEQ, EMB)

